# revision 1
# baseline (speedup 1.0000x reference)
"""2D Daubechies-2 DWT (single level) on Trainium2, 8-core data parallel.

Input  x: [16, 1024, 1024, 1] f32  ->  Output: [16, 512, 512, 4] f32
Channels: [LL, LH, HL, HH] (row-filter, col-filter) per the TF/JAX reference.

Algorithm (per core: 2 images, fully on-chip):
  Both 1-D wavelet passes are PE matmuls with the IMAGE as the stationary
  operand and a banded filter matrix as the moving operand:
      out[m, n] = sum_k lhsT[k, m] * rhs[k, n],  lhsT = image block, rhs = W
  Pass 1 contracts the height axis (k = h-rows, m = w-columns), producing the
  intermediate ALREADY transposed (w on partitions) - which is exactly what
  pass 2 (contracting w) needs as its stationary operand. Pass 2's output
  comes out h'-major, ready for a contiguous channel-interleaved store.
  Zero transpose stages, zero cross-partition shuffles, exact fp32.

  The 1024-long axis is split into 9 overlapping K-blocks (stride 126,
  window 128): a 128-row window covers exactly 63 stride-2 4-tap outputs,
  so no boundary-correction matmuls are needed. Block 0 folds the symmetric
  left-edge mirror into its weights; the right edge needs no padding.
"""
import math

import numpy as np

import concourse.bass as bass
import concourse.tile as tile
from concourse import bacc, mybir
from concourse.bass_utils import run_bass_kernel_spmd

N_CORES = 8
IMGS_PER_CORE = 2
H = W = 1024
HO = WO = 512
IMG_ELEMS = H * W
OUT_ELEMS = HO * WO * 4
F32 = mybir.dt.float32


# ---------------------------------------------------------------- weights ---
def _filters():
    s3 = math.sqrt(3.0)
    den = 4.0 * math.sqrt(2.0)
    h = np.array([(1 + s3) / den, (3 + s3) / den, (3 - s3) / den, (1 - s3) / den],
                 dtype=np.float32)
    g = np.array([h[3], -h[2], h[1], -h[0]], dtype=np.float32)
    return h, g


def make_weights():
    """W_first [128,126], W_mid [128,126], W_last [18,16].
    Columns = [low outs | high outs] halves."""
    h, g = _filters()
    wf = np.zeros((128, 126), dtype=np.float32)
    wf[0, 0] = h[1] + h[2]; wf[1, 0] = h[0] + h[3]
    wf[0, 63] = g[1] + g[2]; wf[1, 63] = g[0] + g[3]
    for m in range(1, 63):
        for k in range(4):
            wf[2 * m - 2 + k, m] = h[k]
            wf[2 * m - 2 + k, 63 + m] = g[k]
    wm = np.zeros((128, 126), dtype=np.float32)
    for m in range(63):
        for k in range(4):
            wm[2 * m + k, m] = h[k]
            wm[2 * m + k, 63 + m] = g[k]
    wl = np.zeros((18, 16), dtype=np.float32)
    for m in range(8):
        for k in range(4):
            wl[2 * m + k, m] = h[k]
            wl[2 * m + k, 8 + m] = g[k]
    return wf, wm, wl


def _wslice(c):
    """(start, width) of overlapping block c along a 1024 axis."""
    if c == 0:
        return 0, 128
    if c == 8:
        return 1006, 18
    return 126 * c - 2, 128


def _sub(ap_full, pcnt, free_off, free_dims):
    """Custom AP on a tile/tensor: pcnt partitions, free offset + [step,count] dims."""
    base = ap_full.ap
    pstep = base[0][0]
    return bass.AP(ap_full.tensor, ap_full.offset + free_off,
                   [[pstep, pcnt]] + [list(d) for d in free_dims])


# ----------------------------------------------------------------- kernel ---
def _build(reps=1):
    nc = bacc.Bacc("TRN2", target_bir_lowering=False, debug=False, num_devices=1)
    xh = nc.dram_tensor("x", [IMGS_PER_CORE * IMG_ELEMS], F32, kind="ExternalInput")
    wfh = nc.dram_tensor("wf", [128, 126], F32, kind="ExternalInput")
    wmh = nc.dram_tensor("wm", [128, 126], F32, kind="ExternalInput")
    wlh = nc.dram_tensor("wl", [18, 16], F32, kind="ExternalInput")
    yh = nc.dram_tensor("y", [IMGS_PER_CORE * OUT_ELEMS], F32, kind="ExternalOutput")
    x_ap = xh.ap()
    y_ap = yh.ap()

    with tile.TileContext(nc) as tc:
        with (
            tc.tile_pool(name="wts", bufs=1) as wpool,
            tc.tile_pool(name="x9", bufs=2) as xpool,
            tc.tile_pool(name="atdt", bufs=2) as adpool,
            tc.tile_pool(name="oq", bufs=4) as opool,
            tc.tile_pool(name="ps", bufs=3, space="PSUM") as pspool,
        ):
            wf = wpool.tile([128, 126], F32, tag="wf")
            wm = wpool.tile([128, 126], F32, tag="wm")
            wl = wpool.tile([18, 16], F32, tag="wl")
            nc.gpsimd.dma_start(wf[:], wfh.ap()[:])
            nc.gpsimd.dma_start(wm[:], wmh.ap()[:])
            nc.gpsimd.dma_start(wl[:], wlh.ap()[:])

            def wtile(b):
                return wf if b == 0 else wm

            for _rep in range(reps):
                for img in range(IMGS_PER_CORE):
                    ib = img * IMG_ELEMS
                    # ---- load x into 9 overlapping 128-row blocks ----
                    x9 = xpool.tile([128, 9 * 1024], F32, tag="x9")
                    # block 0: rows 0..127
                    nc.sync.dma_start(
                        x9[:, 0:1024],
                        bass.AP(xh, ib, [[1024, 128], [1, 1024]]))
                    # blocks 1..7: rows 126b-2 + p
                    nc.sync.dma_start(
                        x9[:, 1024:8 * 1024],
                        bass.AP(xh, ib + 124 * 1024,
                                [[1024, 128], [126 * 1024, 7], [1, 1024]]))
                    # block 8: rows 1006..1023 (18 rows)
                    nc.sync.dma_start(
                        x9[0:18, 8 * 1024:9 * 1024],
                        bass.AP(xh, ib + 1006 * 1024, [[1024, 18], [1, 1024]]))

                    # ---- pass 1: contract h; out = A_t/D_t (w-major) ----
                    ats = []
                    dts = []
                    for c in range(9):
                        wlo, wn = _wslice(c)
                        p1 = pspool.tile([128, 1024], F32, tag="ps")
                        for b in range(8):
                            seg = (b // 4) * 512 + (b % 4) * 126
                            nc.tensor.matmul(
                                p1[0:wn, seg:seg + 126],
                                lhsT=x9[0:128, b * 1024 + wlo: b * 1024 + wlo + wn],
                                rhs=wtile(b)[:, 0:126],
                                start=True, stop=True)
                        # block 8 split into L and H (N=8 each, bank tails)
                        nc.tensor.matmul(
                            p1[0:wn, 504:512],
                            lhsT=x9[0:18, 8 * 1024 + wlo: 8 * 1024 + wlo + wn],
                            rhs=wl[0:18, 0:8], start=True, stop=True)
                        nc.tensor.matmul(
                            p1[0:wn, 1016:1024],
                            lhsT=x9[0:18, 8 * 1024 + wlo: 8 * 1024 + wlo + wn],
                            rhs=wl[0:18, 8:16], start=True, stop=True)

                        at = adpool.tile([128, 512], F32, tag=f"at{c}")
                        dt_ = adpool.tile([128, 512], F32, tag=f"dt{c}")
                        p1f = p1[:]
                        # evac on ACT: L -> at, H -> dt  (per 4-block group)
                        for grp in range(2):
                            nc.scalar.copy(
                                at[0:wn, grp * 252:grp * 252 + 252],
                                _sub(p1f, wn, grp * 512, [[126, 4], [1, 63]]))
                            nc.scalar.copy(
                                dt_[0:wn, grp * 252:grp * 252 + 252],
                                _sub(p1f, wn, grp * 512 + 63, [[126, 4], [1, 63]]))
                        nc.scalar.copy(at[0:wn, 504:512], _sub(p1f, wn, 504, [[1, 8]]))
                        nc.scalar.copy(dt_[0:wn, 504:512], _sub(p1f, wn, 1016, [[1, 8]]))
                        ats.append(at)
                        dts.append(dt_)

                    # ---- pass 2: contract w; out h'-major, interleave to O ----
                    for q in range(4):
                        oq = opool.tile([128, 2048], F32, tag="oq")
                        for inp, ch0 in ((ats, 0), (dts, 1)):
                            p2 = pspool.tile([128, 1024], F32, tag="ps")
                            for c in range(8):
                                seg = (c // 4) * 512 + (c % 4) * 126
                                nc.tensor.matmul(
                                    p2[0:128, seg:seg + 126],
                                    lhsT=inp[c][0:128, 128 * q:128 * q + 128],
                                    rhs=wtile(c)[:, 0:126],
                                    start=True, stop=True)
                            nc.tensor.matmul(
                                p2[0:128, 504:512],
                                lhsT=inp[8][0:18, 128 * q:128 * q + 128],
                                rhs=wl[0:18, 0:8], start=True, stop=True)
                            nc.tensor.matmul(
                                p2[0:128, 1016:1024],
                                lhsT=inp[8][0:18, 128 * q:128 * q + 128],
                                rhs=wl[0:18, 8:16], start=True, stop=True)

                            p2f = p2[:]
                            oqf = oq[:]
                            chL, chH = ch0, ch0 + 2
                            # evac on DVE with channel interleave (stride 4)
                            for grp in range(2):
                                nc.vector.tensor_copy(
                                    _sub(oqf, 128, 1008 * grp + chL,
                                         [[252, 4], [4, 63]]),
                                    _sub(p2f, 128, grp * 512, [[126, 4], [1, 63]]))
                                nc.vector.tensor_copy(
                                    _sub(oqf, 128, 1008 * grp + chH,
                                         [[252, 4], [4, 63]]),
                                    _sub(p2f, 128, grp * 512 + 63, [[126, 4], [1, 63]]))
                            nc.vector.tensor_copy(
                                _sub(oqf, 128, 2016 + chL, [[4, 8]]),
                                _sub(p2f, 128, 504, [[1, 8]]))
                            nc.vector.tensor_copy(
                                _sub(oqf, 128, 2016 + chH, [[4, 8]]),
                                _sub(p2f, 128, 1016, [[1, 8]]))

                        # store: rows h' = 128q + p, 2048 f32 contiguous each
                        nc.sync.dma_start(
                            bass.AP(yh, img * OUT_ELEMS + q * 128 * 2048,
                                    [[2048, 128], [1, 2048]]),
                            oq[:])
    nc.compile()
    return nc


_NC_CACHE = {}


def _get_nc(reps=1):
    if reps not in _NC_CACHE:
        _NC_CACHE[reps] = _build(reps)
    return _NC_CACHE[reps]


def kernel(**inputs):
    x = np.asarray(inputs["x"], dtype=np.float32)
    assert x.shape == (16, 1024, 1024, 1), x.shape
    nc = _get_nc(1)
    wf, wm, wl = make_weights()
    xs = x.reshape(16, IMG_ELEMS).reshape(N_CORES, IMGS_PER_CORE * IMG_ELEMS)
    in_maps = [{"x": xs[i], "wf": wf, "wm": wm, "wl": wl} for i in range(N_CORES)]
    res = run_bass_kernel_spmd(nc, in_maps, core_ids=list(range(N_CORES)))
    out = np.stack([res.results[i]["y"].reshape(IMGS_PER_CORE, HO, WO, 4)
                    for i in range(N_CORES)])
    return out.reshape(16, HO, WO, 4)


def run_reps(reps, n_calls=3):
    """Timing helper: run the reps-variant kernel, return best wall seconds."""
    import time
    nc = _get_nc(reps)
    wf, wm, wl = make_weights()
    rng = np.random.default_rng(0)
    xs = rng.standard_normal((N_CORES, IMGS_PER_CORE * IMG_ELEMS), dtype=np.float32)
    in_maps = [{"x": xs[i], "wf": wf, "wm": wm, "wl": wl} for i in range(N_CORES)]
    best = float("inf")
    for _ in range(n_calls):
        t0 = time.time()
        run_bass_kernel_spmd(nc, in_maps, core_ids=list(range(N_CORES)))
        best = min(best, time.time() - t0)
    return best


# revision 7
# speedup vs baseline: 5.8877x; 5.8877x over previous
"""2D Daubechies-2 DWT (single level) on Trainium2, 8-core data parallel.

Input  x: [16, 1024, 1024, 1] f32  ->  Output: [16, 512, 512, 4] f32
Channels: [LL, LH, HL, HH] = [(wL,hL), (wL,hH), (wH,hL), (wH,hH)].

Per core: 2 images, ~58 device instructions.

Key layout trick: output row index i = 4*p + j (p = partition, j in [0,4)),
so the column pass's source rows h = 2i + delta = 8p + (2j + delta) live inside
partition p's own 8-row band for delta >= 0 — the whole image loads with ONE
contiguous DMA ([p] <- rows 8p..8p+7), taps are free-dim offsets, and only the
j=0 taps of delta<0 need a small one-row-per-partition gather (plus the
symmetric-mirror rows for i=0). Both wavelet passes are Vector-engine FMA
chains (tensor_scalar / scalar_tensor_tensor) that ping-pong between scratch
tiles (never aliasing in1 with out — aliased accumulation is pathologically
slow here). The row pass reads stride-2 free-dim taps from a pitch-1026
mirror-prefixed intermediate and writes the output with channels interleaved;
the store is one fully contiguous DMA per image.
"""
import math

import numpy as np

import concourse.bass as bass
import concourse.tile as tile
from concourse import bacc, mybir
from concourse.bass_utils import run_bass_kernel_spmd

N_CORES = 8
IMGS = 2
IMG_ELEMS = 1024 * 1024
OUT_ELEMS = 512 * 512 * 4
F32 = mybir.dt.float32
AO = mybir.AluOpType
PITCH = 1026  # 2 mirror-prefix cols + 1024 data cols

_S3 = math.sqrt(3.0)
_DEN = 4.0 * math.sqrt(2.0)
H4 = [np.float32((1 + _S3) / _DEN), np.float32((3 + _S3) / _DEN),
      np.float32((3 - _S3) / _DEN), np.float32((1 - _S3) / _DEN)]
G4 = [H4[3], np.float32(-H4[2]), H4[1], np.float32(-H4[0])]
FILT = {"L": H4, "H": G4}


def _ap(handle, offset, dims):
    return bass.AP(handle, offset, [list(d) for d in dims])


def _tap(t, off, dims, pcnt=128, poff=0):
    f = t[:]
    pitch = f.ap[0][0]
    return bass.AP(f.tensor, f.offset + poff * pitch + off,
                   [[pitch, pcnt]] + [list(d) for d in dims])


def _build(reps=1):
    nc = bacc.Bacc("TRN2", target_bir_lowering=False, debug=False, num_devices=1)
    xh = nc.dram_tensor("x", [IMGS * IMG_ELEMS], F32, kind="ExternalInput")
    yh = nc.dram_tensor("y", [IMGS * OUT_ELEMS], F32, kind="ExternalOutput")

    with tile.TileContext(nc) as tc:
        with (
            tc.tile_pool(name="t32", bufs=2) as p32,
            tc.tile_pool(name="t16", bufs=4) as p16,
            tc.tile_pool(name="lh", bufs=1) as plh,
        ):
            for _rep in range(reps):
                # LH: [p, (f 2)(img 2)(j 4)(PITCH)]
                LH = plh.tile([128, 2 * 2 * 4 * PITCH], F32, tag="lh")

                for img in range(IMGS):
                    ib = img * IMG_ELEMS
                    # whole image, contiguous: partition p <- rows 8p..8p+7
                    X8 = p32.tile([128, 8192], F32, tag="t32")
                    nc.sync.dma_start(X8[:], _ap(xh, ib, [[8192, 128], [1, 8192]]))
                    # one-row-per-partition gathers for the j=0 taps (delta<0)
                    xs0 = {}
                    for dlt in (-2, -1):
                        xsd = p16.tile([128, 1024], F32, tag="t16")
                        # partitions 1..127: row 8p + dlt
                        nc.gpsimd.dma_start(
                            _tap(xsd, 0, [[1, 1024]], pcnt=127, poff=1),
                            _ap(xh, ib + (8 + dlt) * 1024, [[8192, 127], [1, 1024]]))
                        # partition 0: mirror row (h=-2 -> row1, h=-1 -> row0)
                        nc.gpsimd.dma_start(
                            _tap(xsd, 0, [[1, 1024]], pcnt=1),
                            _ap(xh, ib + (1 if dlt == -2 else 0) * 1024,
                                [[8192, 1], [1, 1024]]))
                        xs0[dlt] = xsd

                    # column-pass FMA chains, per filter (acc FD = (j 4)(1024))
                    for fi, f in enumerate(("L", "H")):
                        c0, c1, c2, c3 = (float(FILT[f][k]) for k in range(4))
                        a1 = p16.tile([128, 4096], F32, tag="t16")
                        # k=0 (delta=-2): j=0 from xs0[-2]; j=1..3 bands 0,2,4
                        nc.vector.tensor_scalar_mul(
                            _tap(a1, 0, [[1, 1024]]), xs0[-2][:], c0)
                        nc.vector.tensor_scalar_mul(
                            _tap(a1, 1024, [[1024, 3], [1, 1024]]),
                            _tap(X8, 0, [[2048, 3], [1, 1024]]), c0)
                        # k=1 (delta=-1): j=0 from xs0[-1]; j=1..3 bands 1,3,5
                        a2 = p16.tile([128, 4096], F32, tag="t16")
                        nc.vector.scalar_tensor_tensor(
                            out=_tap(a2, 0, [[1, 1024]]), in0=xs0[-1][:], scalar=c1,
                            in1=_tap(a1, 0, [[1, 1024]]), op0=AO.mult, op1=AO.add)
                        nc.vector.scalar_tensor_tensor(
                            out=_tap(a2, 1024, [[1024, 3], [1, 1024]]),
                            in0=_tap(X8, 1024, [[2048, 3], [1, 1024]]), scalar=c1,
                            in1=_tap(a1, 1024, [[1024, 3], [1, 1024]]),
                            op0=AO.mult, op1=AO.add)
                        # k=2 (delta=0): bands 0,2,4,6 — all j
                        a3 = p16.tile([128, 4096], F32, tag="t16")
                        nc.vector.scalar_tensor_tensor(
                            out=_tap(a3, 0, [[1024, 4], [1, 1024]]),
                            in0=_tap(X8, 0, [[2048, 4], [1, 1024]]), scalar=c2,
                            in1=_tap(a2, 0, [[1024, 4], [1, 1024]]),
                            op0=AO.mult, op1=AO.add)
                        # k=3 (delta=1): bands 1,3,5,7 -> LH slice (data at +2)
                        nc.vector.scalar_tensor_tensor(
                            out=_tap(LH, fi * 8 * PITCH + img * 4 * PITCH + 2,
                                     [[PITCH, 4], [1, 1024]]),
                            in0=_tap(X8, 1024, [[2048, 4], [1, 1024]]), scalar=c3,
                            in1=_tap(a3, 0, [[1024, 4], [1, 1024]]),
                            op0=AO.mult, op1=AO.add)

                # row-pass mirror prefix: col0 <- w=1 (data col 3), col1 <- w=0
                nc.vector.tensor_copy(_tap(LH, 0, [[PITCH, 16], [1, 1]]),
                                      _tap(LH, 3, [[PITCH, 16], [1, 1]]))
                nc.vector.tensor_copy(_tap(LH, 1, [[PITCH, 16], [1, 1]]),
                                      _tap(LH, 2, [[PITCH, 16], [1, 1]]))

                # row pass: per (f_in, f_out) FMA chain over stride-2 taps
                Oh = []
                for _img in range(IMGS):
                    oimg = p32.tile([128, 8192], F32, tag="t32")
                    Oh.append(oimg)
                for fi in range(2):
                    for fo, f_out in enumerate(("L", "H")):
                        ch = 2 * fo + fi
                        cs = [float(FILT[f_out][k]) for k in range(4)]
                        acc = None
                        for k in range(3):
                            src = _tap(LH, fi * 8 * PITCH + k,
                                       [[4 * PITCH, 2], [PITCH, 4], [2, 512]])
                            if k == 0:
                                acc = p16.tile([128, 4096], F32, tag="t16")
                                nc.vector.tensor_scalar_mul(
                                    _tap(acc, 0, [[2048, 2], [512, 4], [1, 512]]),
                                    src, cs[0])
                            else:
                                nxt = p16.tile([128, 4096], F32, tag="t16")
                                nc.vector.scalar_tensor_tensor(
                                    out=_tap(nxt, 0, [[2048, 2], [512, 4], [1, 512]]),
                                    in0=src, scalar=cs[k],
                                    in1=_tap(acc, 0, [[2048, 2], [512, 4], [1, 512]]),
                                    op0=AO.mult, op1=AO.add)
                                acc = nxt
                        for img in range(IMGS):
                            nc.vector.scalar_tensor_tensor(
                                out=_tap(Oh[img], ch, [[2048, 4], [4, 512]]),
                                in0=_tap(LH, fi * 8 * PITCH + img * 4 * PITCH + 3,
                                         [[PITCH, 4], [2, 512]]),
                                scalar=cs[3],
                                in1=_tap(acc, img * 2048, [[512, 4], [1, 512]]),
                                op0=AO.mult, op1=AO.add)

                # store: i = 4p + j  ->  fully contiguous per image
                for img in range(IMGS):
                    nc.sync.dma_start(
                        _ap(yh, img * OUT_ELEMS, [[8192, 128], [1, 8192]]),
                        Oh[img][:])
    nc.compile()
    return nc


_NC_CACHE = {}


def _get_nc(reps=1):
    if reps not in _NC_CACHE:
        _NC_CACHE[reps] = _build(reps)
    return _NC_CACHE[reps]


def kernel(**inputs):
    x = np.asarray(inputs["x"], dtype=np.float32)
    assert x.shape == (16, 1024, 1024, 1), x.shape
    nc = _get_nc(1)
    xs = np.ascontiguousarray(x.reshape(N_CORES, IMGS * IMG_ELEMS))
    in_maps = [{"x": xs[i]} for i in range(N_CORES)]
    res = run_bass_kernel_spmd(nc, in_maps, core_ids=list(range(N_CORES)))
    out = np.stack([res.results[i]["y"].reshape(IMGS, 512, 512, 4)
                    for i in range(N_CORES)])
    return out.reshape(16, 512, 512, 4)


def run_reps(reps, n_calls=3):
    import time
    nc = _get_nc(reps)
    rng = np.random.default_rng(0)
    xs = rng.standard_normal((N_CORES, IMGS * IMG_ELEMS), dtype=np.float32)
    in_maps = [{"x": xs[i]} for i in range(N_CORES)]
    best = float("inf")
    for _ in range(n_calls):
        t0 = time.time()
        run_bass_kernel_spmd(nc, in_maps, core_ids=list(range(N_CORES)))
        best = min(best, time.time() - t0)
    return best
